# revision 1
# baseline (speedup 1.0000x reference)
"""Trainium2 Bass kernel for nn_AdaptiveAttention (8-core SPMD).

Sharding: each core owns 2 heads (one 128-dim block of the QKV/head space)
for BOTH batches; outputs are resharded by interleaving: core c produces the
q-rows with (row index) % 8 == c, so each q-block's cross-core exchange is a
small (256KB) per-q-block AllToAll whose slot slicing is static, and the
collectives pipeline behind the attention of later q-blocks.

Per core:
  - Q^T, K^T and V for its 128-dim block (biases, 1/sqrt(dk) and the
    per-batch adaptive score scale folded host-side into weights/biases),
  - per q-block of 512 queries: transposed-score attention S^T = [s, q]
    (lhsT=K^T slice, rhs=Q^T slice; the two heads of a batch row-group-packed
    into one 2-bank PSUM tile and exp'd in a single [128,1024] ACT op; no
    max-subtraction, scores are bounded ~|8|); V is augmented with a ones
    column so the AV matmul's row 64 is the exp row-sum; softmax division is
    applied to the (16x smaller) AV output; the reciprocal runs on all 128
    DVE lanes via a DRAM-transpose bounce; the normalize multiply also
    performs the mod-8 interleave permute so the A2A pushes are contiguous,
  - per-q-block AllToAll (separate DRAM buffers per q-block to avoid false
    WAR serialization; pushes on the gpsimd queue so the sync queue's
    reciprocal/broadcast chain is never delayed),
  - after attention: per q-block Wo + residual + LayerNorm on its 128
    interleaved rows; A2A results for q-blocks 0-2 land during attention, so
    only the last A2A is exposed.

All matmuls in bf16 (fp32 PSUM accumulate); rel err vs the fp32 reference
is ~4e-4 (gate: 2e-2).
"""

import os
import numpy as np
import ml_dtypes

B, T, D = 2, 2048, 1024
H, DK = 16, 64
CPC = 128               # head-dim columns per core (2 heads)
P = 128
NCORES = 8

_BF16 = ml_dtypes.bfloat16

_CACHE = {}
LAST_RESULTS = None


def _sinusoidal_pe(max_len, d_model):
    pos = np.arange(max_len)[:, None].astype(np.float32)
    div = np.exp(np.arange(0, d_model, 2).astype(np.float32) * (-np.log(10000.0) / d_model))
    pe = np.zeros((max_len, d_model), dtype=np.float32)
    pe[:, 0::2] = np.sin(pos * div)
    pe[:, 1::2] = np.cos(pos * div)
    return pe


def _build(taps=False):
    """Build + compile the SPMD Bass graph (one NEFF, runs on all 8 cores)."""
    import concourse.bass as bass
    import concourse.mybir as mybir
    import concourse.tile as tile
    from concourse import bacc

    f32 = mybir.dt.float32
    bf = mybir.dt.bfloat16
    Exp = mybir.ActivationFunctionType.Exp
    Sqrt = mybir.ActivationFunctionType.Sqrt
    sub = mybir.AluOpType.subtract
    mult = mybir.AluOpType.mult

    nc = bacc.Bacc("TRN2", target_bir_lowering=False, debug=False, num_devices=NCORES)

    xt_d = nc.dram_tensor("xt", [B, D, T], bf, kind="ExternalInput")
    xres_d = nc.dram_tensor("xres", [4, P, D], f32, kind="ExternalInput")
    wq_d = nc.dram_tensor("wq", [D, CPC], bf, kind="ExternalInput")
    wk_d = nc.dram_tensor("wk", [B, D, CPC], bf, kind="ExternalInput")
    wv_d = nc.dram_tensor("wv", [D, CPC], bf, kind="ExternalInput")
    bq_d = nc.dram_tensor("bq", [B, CPC], f32, kind="ExternalInput")
    bk_d = nc.dram_tensor("bk", [B, CPC], f32, kind="ExternalInput")
    bv_d = nc.dram_tensor("bv", [B, CPC], f32, kind="ExternalInput")
    wo_d = nc.dram_tensor("wo", [D, D], bf, kind="ExternalInput")
    lng_d = nc.dram_tensor("lng", [D], f32, kind="ExternalInput")
    lnb_d = nc.dram_tensor("lnb", [D], f32, kind="ExternalInput")
    out_d = nc.dram_tensor("out", [4, P, D], f32, kind="ExternalOutput")
    a2ai_d = [nc.dram_tensor(f"a2ai{i}", [NCORES, CPC, B, 2, 64], bf, kind="Internal")
              for i in range(2)]
    a2ao_d = [nc.dram_tensor(f"a2ao{i}", [NCORES, CPC, B, 2, 64], bf, kind="Internal")
              for i in range(2)]
    tap_d = {}
    if taps:
        tap_d["qt"] = nc.dram_tensor("tap_qt", [P, B, T], bf, kind="ExternalOutput")
        tap_d["kt"] = nc.dram_tensor("tap_kt", [P, B, T], bf, kind="ExternalOutput")
        tap_d["v"] = nc.dram_tensor("tap_v", [P, B, 16, 2, 65], bf, kind="ExternalOutput")
        tap_d["att"] = nc.dram_tensor("tap_att", [64, B, 2, 4, 512], bf, kind="ExternalOutput")
        tap_d["rs"] = nc.dram_tensor("tap_rs", [4, 4, 512], f32, kind="ExternalOutput")

    def bcast_ap(src, nparts):
        """Partition-broadcast DMA source AP from a 1-partition AP."""
        return bass.AP(
            tensor=src.tensor,
            offset=src.offset,
            ap=[[0, nparts]] + [list(d) for d in src.ap[1:]],
        )

    with tile.TileContext(nc) as tc:
        with tc.tile_pool(name="const", bufs=1) as const:
            # ---- load inputs (weights first so matmuls start early;
            # spread across the sync/scalar/gpsimd DMA queues) ----
            qeng = [nc.sync, nc.scalar, nc.gpsimd]
            # per-(batch, k-tile) input tiles so the first projection
            # matmuls fire as soon as their own 512KB slice lands, instead
            # of waiting on the whole 4MB load (coarse region tracking)
            xt_sb = [[const.tile([P, T], bf, name=f"xt{b}_{k}")
                      for k in range(8)] for b in range(B)]
            xt_ap = xt_d.ap()
            wq_sb = const.tile([P, 8, CPC], bf)
            wk_sb = const.tile([P, B, 8, CPC], bf)
            wv_sb = const.tile([P, 8, CPC], bf)
            for k in range(8):
                qeng[k % 3].dma_start(out=wq_sb[:, k, :], in_=wq_d.ap()[k * P:(k + 1) * P, :])
            for b in range(B):
                for k in range(8):
                    qeng[k % 3].dma_start(
                        out=wk_sb[:, b, k, :], in_=wk_d.ap()[b, k * P:(k + 1) * P, :]
                    )
            for k in range(8):
                qeng[k % 3].dma_start(out=xt_sb[0][k][:], in_=xt_ap[0, k * P:(k + 1) * P, :])
            for k in range(8):
                qeng[k % 3].dma_start(out=xt_sb[1][k][:], in_=xt_ap[1, k * P:(k + 1) * P, :])
            for k in range(8):
                qeng[k % 3].dma_start(out=wv_sb[:, k, :], in_=wv_d.ap()[k * P:(k + 1) * P, :])

            wo_sb = const.tile([P, 8, D], bf)
            wo_ap = wo_d.ap()
            for k in range(8):
                qeng[k % 3].dma_start(out=wo_sb[:, k, :], in_=wo_ap[k * P:(k + 1) * P, :])

            bq_sb = const.tile([P, B], f32)
            bk_sb = const.tile([P, B], f32)
            nc.sync.dma_start(out=bq_sb[:], in_=bq_d.ap().rearrange("b p -> p b"))
            nc.sync.dma_start(out=bk_sb[:], in_=bk_d.ap().rearrange("b p -> p b"))
            bv_bc = const.tile([P, B, CPC], f32)
            for b in range(B):
                nc.sync.dma_start(out=bv_bc[:, b, :], in_=bcast_ap(bv_d.ap()[b:b + 1, :], P))
            lng_b = const.tile([P, D], f32)
            lnb_b = const.tile([P, D], f32)
            nc.sync.dma_start(out=lng_b[:], in_=bcast_ap(lng_d.ap()[None, :], P))
            nc.sync.dma_start(out=lnb_b[:], in_=bcast_ap(lnb_d.ap()[None, :], P))

            eps_sb = const.tile([P, 1], f32)
            nc.vector.memset(eps_sb[:], 1e-5)

            QT_sb = const.tile([P, B, T], bf)
            KT_sb = const.tile([P, B, T], bf)
            # V augmented with a ones-column per head (col 64 == 1.0) so the
            # AV matmul's output row 64 is the exp row-sum.
            V_sb = const.tile([P, B, 16, 2, 65], bf)
            nc.vector.memset(V_sb[:], 1.0)
            # normalized attention, permuted so that the per-slot A2A pushes
            # are contiguous: free index inside a q-block is j*64+i for the
            # q-row qb*512 + j + 8*i (slot j gets rows == j mod 8).
            # One tile per q-block so a later block's normalize never picks up
            # a false WAR against an earlier block's (collective-queued) pushes.
            attT_sb = [const.tile([64, B, 2, 512], bf, name=f"attT{i}")
                       for i in range(4)]

            # ---- phase 1: projections ----
            with tc.tile_pool(name="qk_ps", bufs=4, space="PSUM") as qk_ps, \
                 tc.tile_pool(name="v_ps", bufs=4, space="PSUM") as v_ps:
                for wname, b_sb, dst in (("q", bq_sb, QT_sb), ("k", bk_sb, KT_sb)):
                    for b in range(B):
                        for n in range(4):
                            ps = qk_ps.tile([P, 512], f32, tag="qkps")
                            for k in range(8):
                                lhsT = (wq_sb[:, k, :] if wname == "q"
                                        else wk_sb[:, b, k, :])
                                nc.tensor.matmul(
                                    ps[:],
                                    lhsT=lhsT,
                                    rhs=xt_sb[b][k][:, n * 512:(n + 1) * 512],
                                    start=(k == 0), stop=(k == 7),
                                )
                            nc.vector.tensor_scalar_add(
                                out=dst[:, b, n * 512:(n + 1) * 512],
                                in0=ps[:], scalar1=b_sb[:, b:b + 1],
                            )
                for b in range(B):
                    for mt in range(16):
                        ps = v_ps.tile([P, CPC], f32, tag="vps")
                        for k in range(8):
                            nc.tensor.matmul(
                                ps[:],
                                lhsT=xt_sb[b][k][:, mt * P:(mt + 1) * P],
                                rhs=wv_sb[:, k, :],
                                start=(k == 0), stop=(k == 7),
                            )
                        for hh in range(2):
                            hsl = slice(64 * hh, 64 * (hh + 1))
                            nc.vector.tensor_add(
                                out=V_sb[:, b, mt, hh, 0:64],
                                in0=ps[:, hsl], in1=bv_bc[:, b, hsl],
                            )

            # ---- phase 2: attention (S^T layout), per-qb AllToAll ----
            with tc.tile_pool(name="sp_ps", bufs=2, space="PSUM") as sp_ps, \
                 tc.tile_pool(name="av_ps", bufs=4, space="PSUM") as av_ps, \
                 tc.tile_pool(name="et_pool", bufs=6) as et_pool, \
                 tc.tile_pool(name="avs_pool", bufs=8) as avs_pool, \
                 tc.tile_pool(name="rc_pool", bufs=4) as rc_pool, \
                 tc.tile_pool(name="dram_pool", bufs=4, space="DRAM") as dram_pool, \
                 tc.tile_pool(name="rb_pool", bufs=4) as rb_pool:
                for qb in range(4):
                    qsl = slice(qb * 512, (qb + 1) * 512)
                    avs = [av_ps.tile([65, 512], f32, tag="av", name=f"av{qb}_{u}")
                           for u in range(4)]
                    for sc in range(16):
                        for b in range(B):
                            # one 2-bank PSUM tile holds both heads' scores;
                            # the two matmuls use distinct row groups (h0 rows
                            # 0-63, h1 rows 64-127) and run concurrently.
                            sp = sp_ps.tile([P, 2, 512], f32, tag="sp")
                            nc.tensor.matmul(
                                sp[:, 0, :],
                                lhsT=KT_sb[0:64, b, sc * P:(sc + 1) * P],
                                rhs=QT_sb[0:64, b, qsl],
                                start=True, stop=True,
                            )
                            nc.tensor.matmul(
                                sp[:, 1, :],
                                lhsT=KT_sb[64:128, b, sc * P:(sc + 1) * P],
                                rhs=QT_sb[64:128, b, qsl],
                                start=True, stop=True,
                            )
                            et = et_pool.tile([P, 2, 512], bf, tag="et")
                            nc.scalar.activation(out=et[:], in_=sp[:], func=Exp)
                            for hh in range(2):
                                nc.tensor.matmul(
                                    avs[2 * b + hh][:],
                                    lhsT=V_sb[:, b, sc, hh, :],
                                    rhs=et[:, hh, :],
                                    start=(sc == 0), stop=(sc == 15),
                                )
                    # epilogue: drain PSUM to SBUF, transpose-trick reciprocal,
                    # broadcast, normalize (+interleave-permute), push, A2A.
                    av_sb = [avs_pool.tile([65, 512], f32, tag="avsb", name=f"avsb{qb}_{u}")
                             for u in range(4)]
                    for u in range(4):
                        nc.vector.tensor_copy(out=av_sb[u][:], in_=avs[u][:])
                    rsd = dram_pool.tile([4, 512], f32, tag="rsd")
                    for u in range(4):
                        nc.sync.dma_start(out=rsd[u:u + 1, :], in_=av_sb[u][64:65, :])
                    rst = rc_pool.tile([P, 16], f32, tag="rst")
                    nc.sync.dma_start(
                        out=rst[:], in_=rsd[:].rearrange("u (a p) -> p (u a)", p=P)
                    )
                    rct = rc_pool.tile([P, 16], f32, tag="rct")
                    nc.vector.reciprocal(out=rct[:], in_=rst[:])
                    rcd = dram_pool.tile([4, 512], f32, tag="rcd")
                    nc.sync.dma_start(
                        out=rcd[:].rearrange("u (a p) -> p (u a)", p=P), in_=rct[:]
                    )
                    if taps:
                        nc.sync.dma_start(out=tap_d["rs"].ap()[qb], in_=rcd[:])
                    for b in range(B):
                        for hh in range(2):
                            u = 2 * b + hh
                            rb = rb_pool.tile([64, 512], f32, tag="rb")
                            nc.sync.dma_start(out=rb[:], in_=bcast_ap(rcd[u:u + 1, :], 64))
                            # normalize + interleave-permute in one DVE op
                            nc.vector.tensor_mul(
                                out=attT_sb[qb][:, b, hh, :].rearrange("p (j i) -> p i j", j=8),
                                in0=av_sb[u][0:64, :].rearrange("p (i j) -> p i j", j=8),
                                in1=rb[:].rearrange("p (i j) -> p i j", j=8),
                            )
                    # contiguous per-slot pushes into the q-block-pair A2A
                    # input (gpsimd queue so the burst never delays the sync
                    # queue's reciprocal/broadcast chain of the next q-block)
                    for j in range(NCORES):
                        for b in range(B):
                            dst = a2ai_d[qb // 2].ap()[j].rearrange(
                                "(hh p) bb qq q -> p hh bb qq q", hh=2)[:, :, b, qb % 2, :]
                            nc.gpsimd.dma_start(
                                out=dst,
                                in_=attT_sb[qb][:, b, :, j * 64:(j + 1) * 64],
                            )
                    if qb % 2 == 1:
                        # one 512KB AllToAll per q-block pair: half the
                        # latency floors of per-qb collectives
                        nc.gpsimd.collective_compute(
                            "AllToAll",
                            mybir.AluOpType.bypass,
                            replica_groups=[list(range(NCORES))],
                            ins=[a2ai_d[qb // 2].ap()],
                            outs=[a2ao_d[qb // 2].ap()],
                        )

            # ---- phase 3: interleaved-row Wo + residual + LN per q-block ----
            with tc.tile_pool(name="wo_ps", bufs=2, space="PSUM") as wo_ps, \
                 tc.tile_pool(name="attf_pool", bufs=2) as attf_pool, \
                 tc.tile_pool(name="y_pool", bufs=2) as y_pool, \
                 tc.tile_pool(name="xr_pool", bufs=2) as xr_pool, \
                 tc.tile_pool(name="stat", bufs=4) as stat:
                for qb in range(4):
                    attf = attf_pool.tile([P, 8, B, 64], bf, tag="attf")
                    for k in range(8):
                        nc.sync.dma_start(
                            out=attf[:, k, :, :],
                            in_=a2ao_d[qb // 2].ap()[k][:, :, qb % 2, :],
                        )
                    y = y_pool.tile([P, D], f32, tag="y")
                    xr = xr_pool.tile([P, D], f32, tag="xr")
                    nc.sync.dma_start(out=xr[:], in_=xres_d.ap()[qb])
                    for n in range(2):
                        nsl = slice(n * 512, (n + 1) * 512)
                        ps = wo_ps.tile([P, 512], f32, tag="wops")
                        for k in range(8):
                            nc.tensor.matmul(
                                ps[:],
                                lhsT=attf[:, k, :, :],
                                rhs=wo_sb[:, k, nsl],
                                start=(k == 0), stop=(k == 7),
                            )
                        nc.vector.tensor_add(out=y[:, nsl], in0=ps[:], in1=xr[:, nsl])
                    st = stat.tile([P, 2, 6], f32, tag="st")
                    nc.vector.bn_stats(out=st[:, 0, :], in_=y[:, 0:512])
                    nc.vector.bn_stats(out=st[:, 1, :], in_=y[:, 512:1024])
                    mv = stat.tile([P, 2], f32, tag="mv")
                    nc.vector.bn_aggr(out=mv[:], in_=st[:])
                    std = stat.tile([P, 1], f32, tag="std")
                    nc.scalar.activation(out=std[:], in_=mv[:, 1:2], func=Sqrt, bias=eps_sb[:])
                    rstd = stat.tile([P, 1], f32, tag="rstd")
                    nc.vector.reciprocal(out=rstd[:], in_=std[:])
                    nc.vector.tensor_scalar(
                        out=y[:], in0=y[:], scalar1=mv[:, 0:1], scalar2=rstd[:],
                        op0=sub, op1=mult,
                    )
                    nc.vector.tensor_mul(out=y[:], in0=y[:], in1=lng_b[:])
                    nc.vector.tensor_add(out=y[:], in0=y[:], in1=lnb_b[:])
                    nc.sync.dma_start(out=out_d.ap()[qb], in_=y[:])

            if taps:
                for b in range(B):
                    nc.sync.dma_start(out=tap_d["qt"].ap()[:, b, :], in_=QT_sb[:, b, :])
                    nc.sync.dma_start(out=tap_d["kt"].ap()[:, b, :], in_=KT_sb[:, b, :])
                    for hh in range(2):
                        for qbi in range(4):
                            nc.sync.dma_start(
                                out=tap_d["att"].ap()[:, b, hh, qbi, :],
                                in_=attT_sb[qbi][:, b, hh, :],
                            )
                    for mt in range(16):
                        nc.sync.dma_start(
                            out=tap_d["v"].ap()[:, b, mt, :, :], in_=V_sb[:, b, mt, :, :]
                        )

    nc.compile()
    return nc


def _prep_inputs(x, quantile, quantile_importance,
                 Wq, bq, Wk, bk, Wv, bv, Wo, bo,
                 qpq_w1, qpq_b1, qpq_w2, qpq_b2,
                 qpk_w1, qpk_b1, qpk_w2, qpk_b2,
                 qpv_w1, qpv_b1, qpv_w2, qpv_b2,
                 ln_g, ln_b):
    pe = _sinusoidal_pe(T, D)
    xp = x.astype(np.float32) + pe[None]

    q = quantile.astype(np.float32)

    def mlp(w1, b1, w2, b2):
        return np.maximum(q @ w1 + b1, 0.0) @ w2 + b2

    q_embed = mlp(qpq_w1, qpq_b1, qpq_w2, qpq_b2)
    k_embed = mlp(qpk_w1, qpk_b1, qpk_w2, qpk_b2)
    v_embed = mlp(qpv_w1, qpv_b1, qpv_w2, qpv_b2)

    buf = quantile_importance.astype(np.float32)
    idx = np.clip((q[:, 0] * 100).astype(np.int32), 0, 99)
    mx = buf.max()
    imp = buf[idx]
    imp = np.where(mx > 0, imp / mx, imp)
    scales = (1.0 + imp).astype(np.float32)          # [B], folded into Wk/bk
    rdk = np.float32(1.0 / np.sqrt(DK))              # folded into Wq/bq

    xt_all = np.ascontiguousarray(np.transpose(xp, (0, 2, 1))).astype(_BF16)  # [B, D, T]
    xpb = xp + bo[None, None, :]
    in_maps = []
    for c in range(NCORES):
        cols = slice(c * CPC, (c + 1) * CPC)
        # interleaved residual rows: xres[qb, b*64+jj] = (x+pe+bo)[b, qb*512 + jj*8 + c]
        xres = np.empty((4, P, D), np.float32)
        for qbi in range(4):
            for b in range(B):
                xres[qbi, b * 64:(b + 1) * 64] = xpb[b, qbi * 512 + c: (qbi + 1) * 512: 8]
        in_maps.append({
            "xt": xt_all,
            "xres": xres,
            "wq": np.ascontiguousarray(Wq[:, cols] * rdk).astype(_BF16),
            "wk": np.ascontiguousarray(Wk[None, :, cols] * scales[:, None, None]).astype(_BF16),
            "wv": np.ascontiguousarray(Wv[:, cols]).astype(_BF16),
            "bq": np.ascontiguousarray((bq[None, cols] + q_embed[:, cols]) * rdk).astype(np.float32),
            "bk": np.ascontiguousarray((bk[None, cols] + k_embed[:, cols]) * scales[:, None]).astype(np.float32),
            "bv": np.ascontiguousarray(bv[None, cols] + v_embed[:, cols]).astype(np.float32),
            "wo": Wo.astype(_BF16),
            "lng": ln_g.astype(np.float32),
            "lnb": ln_b.astype(np.float32),
        })
    return in_maps


def kernel(**inputs):
    global LAST_RESULTS
    from concourse import bass_utils

    inputs = {k: np.asarray(v) for k, v in inputs.items()}
    if "nc" not in _CACHE:
        _CACHE["nc"] = _build()
    nc = _CACHE["nc"]

    in_maps = _prep_inputs(**inputs)
    res = bass_utils.run_bass_kernel_spmd(nc, in_maps, core_ids=list(range(NCORES)))
    LAST_RESULTS = res

    out = np.zeros((B, T, D), np.float32)
    for c in range(NCORES):
        o = res.results[c]["out"]  # [4, 128, D]; row = b*64+jj
        for qbi in range(4):
            for b in range(B):
                out[b, qbi * 512 + c:(qbi + 1) * 512:8, :] = o[qbi, b * 64:(b + 1) * 64]
    return out

